# revision 6
# baseline (speedup 1.0000x reference)
"""DCGNN forward kernel for 8 Trainium2 NeuronCores.

The reference network is linear in x (the adjacency is built only from
coord), and the final output is just [B, 2].  The entire pipeline
  x -> Chebyshev(L) -> cheb_W -> (+cheb_b) -> 1x1 conv affine -> FC
therefore collapses to a single affine map

    out[b, n] = sum_k x_flat[b, k] * G[k, n] + const[n],

with G = [C*F_IN, NCLS] = [31744, 2] precomputed on the host from the
tiny parameter tensors (~0.2 MFLOP in f64).  The device kernel is a pure
memory-bound streaming matmul: each core reads its batch shard of x
exactly once.

Per-core device pipeline (data-parallel over batch, no collectives):
  - Host pre-transposes and pre-casts the shard to bf16 xT [128, KT*256]
    (partition-major k-tiles), halving HBM traffic vs f32 and removing
    all on-device transposes.
  - DMA xT in big contiguous chunks (~2 MB) on the sync (HWDGE) queue;
    in-order completion lets matmuls start after the first chunk.
  - PE matmul accumulate: acc[2, 256] += G_tile[128, 2].T @ xT_tile[128, 256]
    (bf16 x bf16 -> fp32 PSUM), 248 k-tiles, weights stationary 2 cols
    so LDWEIGHTS is ~free.
  - DVE copy PSUM -> SBUF, DMA out [2, 256].
"""

import numpy as np

_B, _C, _F_IN, _NCLS = 2048, 62, 512, 2
_THRESH = 0.1
_NCORES = 8
_B_LOC = _B // _NCORES            # 256
_KDIM = _C * _F_IN                # 31744
_P = 128
_KT = _KDIM // _P                 # 248 k-tiles
_CHUNK_KT = 31                    # k-tiles per DMA chunk
_NCHUNK = _KT // _CHUNK_KT        # 8

# x on-wire dtype: "bfloat16" (rel err ~2.5e-3) or "float8e4" (halves HBM
# traffic again; host-side noise-shaped quantization keeps rel err ~2.6e-3,
# see _ef_quantize).  G always stays bf16 (its fp8 error cannot be shaped).
_X_DTYPE = "bfloat16"


def _precompute_g(coord, adj_w1, adj_b1, adj_w2, adj_b2, cheb_W, cheb_b,
                  conv_w, conv_b, fc_w, fc_b):
    """Fold every parameter into G [KDIM, NCLS] and const [NCLS].

    The adjacency MLP + threshold is done in f32 to mirror the reference
    bit-for-bit (the > 0.1 threshold must see the same values); the
    Laplacian / Chebyshev / folding run in f64 for accuracy.
    """
    f32 = np.float32
    coord = coord.astype(f32)
    h = np.maximum(coord @ adj_w1.astype(f32) + adj_b1.astype(f32), f32(0))
    w_star = (h @ adj_w2.astype(f32) + adj_b2.astype(f32))[..., 0]   # [C, C]

    C = w_star.shape[0]
    wd = w_star.astype(np.float64)
    eye = np.eye(C, dtype=bool)
    A = np.where((wd > _THRESH) & ~eye, wd, 0.0)
    deg = A.sum(axis=1)
    dis = np.where(deg > 0, 1.0 / np.sqrt(np.where(deg > 0, deg, 1.0)), 0.0)
    L = -(dis[:, None] * A * dis[None, :])

    K = cheb_W.shape[0]
    T = np.zeros((K, C, C))
    T[0] = np.eye(C)
    T[1] = L
    for k in range(2, K):
        T[k] = 2.0 * (L @ T[k - 1]) - T[k - 2]

    ncls = fc_w.shape[1]
    Fc = fc_w.astype(np.float64).reshape(C, -1, ncls)               # [C, F_OUT, N]
    cw = float(np.asarray(conv_w).reshape(-1)[0])
    cb = float(np.asarray(conv_b).reshape(-1)[0])

    G = np.zeros((C, cheb_W.shape[1], ncls))
    for k in range(K):
        U = np.einsum('if,cfn->icn', cheb_W[k].astype(np.float64), Fc,
                      optimize=True)
        G += np.einsum('cj,icn->jin', T[k], U, optimize=True)
    G *= cw

    const = ((cw * np.tile(cheb_b.astype(np.float64), C) + cb)
             @ fc_w.astype(np.float64)) + fc_b.astype(np.float64)
    return G.reshape(C * cheb_W.shape[1], ncls), const.astype(f32)


_NC_CACHE = {}


def _build_nc(reps=1):
    """Build the bass module. reps>1 emits the whole pipeline that many
    times back-to-back (same I/O) — used only for steady-state timing."""
    key = (reps, _X_DTYPE)
    if key in _NC_CACHE:
        return _NC_CACHE[key]

    import concourse.mybir as mybir
    import concourse.tile as tile
    from concourse import bacc

    f32 = mybir.dt.float32
    bf16 = mybir.dt.bfloat16
    xdt = getattr(mybir.dt, _X_DTYPE)

    # Bacc (not plain Bass): its finalize() runs the TRN2 sync-wait
    # legalization (split >1-wait instructions, move matmul waits to
    # LDWEIGHTS) that walrus codegen requires.
    nc = bacc.Bacc()
    # xT layout: element (p, t*B_LOC + b) = x_shard[b, t*128 + p]
    x_dram = nc.declare_dram_parameter("xt", [_P, _KT * _B_LOC], xdt,
                                       isOutput=False)
    g_dram = nc.declare_dram_parameter("g", [_P, _KT * _NCLS], bf16,
                                       isOutput=False)
    out_dram = nc.declare_dram_parameter("out_t", [_NCLS, _B_LOC], f32,
                                         isOutput=True)

    with tile.TileContext(nc) as tc:
        with (
            tc.tile_pool(name="const", bufs=1) as const_pool,
            # all chunks SBUF-resident: every DMA can be issued at pass
            # start and drain in FIFO order with zero recycle stalls
            tc.tile_pool(name="x", bufs=_NCHUNK) as x_pool,
            tc.tile_pool(name="acc", bufs=1, space="PSUM") as acc_pool,
        ):
            g_sb = const_pool.tile([_P, _KT * _NCLS], bf16, tag="g")
            nc.sync.dma_start(out=g_sb[:], in_=g_dram[:])

            def one_pass():
                acc = acc_pool.tile([_NCLS, _B_LOC], f32)
                for c in range(_NCHUNK):
                    xc = x_pool.tile([_P, _CHUNK_KT * _B_LOC], xdt, tag="x")
                    lo = c * _CHUNK_KT * _B_LOC
                    nc.sync.dma_start(
                        out=xc[:], in_=x_dram[:, lo:lo + _CHUNK_KT * _B_LOC])
                    for s in range(_CHUNK_KT):
                        kt = c * _CHUNK_KT + s
                        nc.tensor.matmul(
                            acc[:],
                            g_sb[:, kt * _NCLS:(kt + 1) * _NCLS],
                            xc[:, s * _B_LOC:(s + 1) * _B_LOC],
                            start=(kt == 0), stop=(kt == _KT - 1))

                out_sb = const_pool.tile([_NCLS, _B_LOC], f32, tag="out")
                nc.vector.tensor_copy(out_sb[:], acc[:])
                nc.sync.dma_start(out=out_dram[:], in_=out_sb[:])

            for _rep in range(reps):
                one_pass()

    # Bacc.finalize runs the legalization pipeline (sync-wait splitting,
    # matmul->LDWEIGHTS wait moves, register allocation).
    nc.finalize()

    _NC_CACHE[key] = nc
    return nc


def _snake_perm(g64):
    """Boustrophedon ordering of the K rows of G through the (g0, g1)
    plane: ~sqrt(K) strips by g0, serpentine by g1 within each strip.
    Consecutive rows end up close in G-space, which is what makes the
    error-feedback quantization noise cancel in the dot products."""
    nstrip = int(round(np.sqrt(g64.shape[0] / 1.0)))
    order0 = np.argsort(g64[:, 0], kind="stable")
    parts = []
    for si, s in enumerate(np.array_split(order0, nstrip)):
        s2 = s[np.argsort(g64[s, 1], kind="stable")]
        parts.append(s2[::-1] if si % 2 else s2)
    return np.concatenate(parts)


def _ef_quantize(xp, fp8):
    """First-order noise-shaped (error-feedback / sigma-delta) e4m3
    quantization along axis 1.  The quantization residual of column j is
    carried into column j+1, so the noise seen by a dot product against
    a SMOOTH sequence g[j] telescopes: err = sum_j e_j (g[j+1]-g[j]).
    With the snake ordering |g[j+1]-g[j]| ~ |g|/30, this measures ~10x
    lower output error than round-to-nearest (2.6e-3 vs 2.7e-2)."""
    xq = np.empty(xp.shape, dtype=fp8)
    e = np.zeros(xp.shape[0], np.float32)
    for j in range(xp.shape[1]):
        v = xp[:, j] + e
        q = v.astype(fp8)
        e = v - q.astype(np.float32)
        xq[:, j] = q
    return xq


def _prepare_in_maps(x, coord, adj_w1, adj_b1, adj_w2, adj_b2, cheb_W,
                     cheb_b, conv_w, conv_b, fc_w, fc_b):
    import ml_dtypes

    g64, const = _precompute_g(coord, adj_w1, adj_b1, adj_w2, adj_b2,
                               cheb_W, cheb_b, conv_w, conv_b, fc_w, fc_b)
    x_flat = np.asarray(x, dtype=np.float32).reshape(_B, _KDIM)

    if _X_DTYPE == "float8e4":
        xdt = ml_dtypes.float8_e4m3
        perm = _snake_perm(g64)
        g64 = g64[perm]
        x_flat = _ef_quantize(x_flat[:, perm], xdt)
    else:
        xdt = ml_dtypes.bfloat16

    # Device layout: g_host[p, t*NCLS + n] = G[t*128 + p, n]
    g_host = np.ascontiguousarray(
        g64.reshape(_KT, _P, _NCLS).transpose(1, 0, 2).reshape(_P, -1)
    ).astype(ml_dtypes.bfloat16)

    in_maps = []
    for i in range(_NCORES):
        shard = x_flat[i * _B_LOC:(i + 1) * _B_LOC]          # [256, KDIM]
        # xt[p, t*B_LOC + b] = shard[b, t*128 + p]
        xt = np.ascontiguousarray(
            shard.reshape(_B_LOC, _KT, _P).transpose(2, 1, 0).reshape(
                _P, _KT * _B_LOC)).astype(xdt)
        in_maps.append({"xt": xt, "g": g_host})
    return in_maps, const


def kernel(x, coord, adj_w1, adj_b1, adj_w2, adj_b2, cheb_W, cheb_b,
           conv_w, conv_b, fc_w, fc_b):
    from concourse.bass_utils import run_bass_kernel_spmd

    in_maps, const = _prepare_in_maps(
        x, coord, adj_w1, adj_b1, adj_w2, adj_b2, cheb_W, cheb_b,
        conv_w, conv_b, fc_w, fc_b)

    nc = _build_nc()
    res = run_bass_kernel_spmd(nc, in_maps, core_ids=list(range(_NCORES)))
    global _LAST_RESULTS
    _LAST_RESULTS = res

    out = np.concatenate([r["out_t"].T for r in res.results], axis=0)
    return (out + const[None, :]).astype(np.float32)


_LAST_RESULTS = None
